# revision 1
# baseline (speedup 1.0000x reference)
"""Trainium2 Bass kernel for nn_Attn_48052094107916 (sparse_attention).

Math (per batch b):
  q = x @ Wq.T -> [N, 4, 16];  k = x @ Wk.T -> [N, 4, 16];  v = x @ Wv.T -> [N, 8, 16]
  attn[g,i,j] = <q[i,g,:], k[j,g,:]>
  mw[i,j,g,l] = (masks @ mask_proj)[i,j,g*8+l]
  scores[l,i,j] = sum_g attn[g,i,j] * mw[i,j,g,l]
  out[i,l,:]  = softmax_j(scores[l,i,:]) @ v[:,l,:]

Key restructuring: using mask_proj's rank-3 structure,
  scores[l] = sum_m masks_m ⊙ w_{m,l},   w_{m,l} = sum_g P[m,g,l] attn_g
and w is computed DIRECTLY on the TensorEngine by scaling q into 24 virtual
heads: w[m,l][j,i] = <k[j,:], qtilde[m,l][i,:]> with
qtilde[m,l][i,gd] = P[m,g,l] q[i,gd].  This moves the whole 4->8 head mixing
contraction onto the PE; the vector engines only do 3 masked products + 2
adds + exp per score plane.

Sharding: 8 cores, core r owns query rows [128r, 128r+128) for ALL batches
(sequence parallel).  Every core gets the full x (transposed + fp16-cast on
host, for K/V), its own slice of masks (pre-transposed on host), plus small
host-precomputed tensors.  No collectives.

Score planes kept transposed [j, i] so PV needs no transpose; softmax
denominator via an extra all-ones column in the PV stationary operand; no
max subtraction needed (|scores| < ~25, safe in f32 exp).  fp16 is used on
the whole scores chain (absolute accuracy matters before exp); probs/v in
bf16 (only relative accuracy needed after exp).  The final [17,(l,i)] PV
accumulator is turned into [i,(l,d)] via a single DVE 32x32 block
transpose (the block permutation is absorbed into the output DMA's access
pattern).

Engine balance per (b, key-chunk) iteration (cost-model):  PE: 6 w-matmuls
+ 8 PV matmuls; ACT: 2 w-copies + exp + v17 copies; DVE: 1 w-copy +
coupling mult + add-pairs on 1/3 of iterations; GPSIMD: add-pairs on 2/3
of iterations + 2/3 of the q-scaling + memsets.  Both adds of an
iteration stay on one engine (avoids a cross-engine hop in the serial
mult->add->add->exp chain).  Modeled per-core exec ~287 us
(DVE 236 / ACT 233 / GPSIMD 222 / PE 121 busy).
"""

import os
import sys

import numpy as np

sys.path.insert(0, "/opt/trn_rl_repo")

B, N, C = 8, 1024, 128
G, L, HD = 4, 8, 16
NCORES = 8
RQ = N // NCORES  # query rows per core = 128
NCH = N // 128  # key chunks = 8

_cache = {}


def _build():
    import concourse.bacc as bacc
    import concourse.bass as bass
    import concourse.tile as tile
    from concourse import mybir

    f32 = mybir.dt.float32
    bf16 = mybir.dt.bfloat16
    fp16 = mybir.dt.float16
    AF = mybir.ActivationFunctionType
    OP = mybir.AluOpType

    nc = bacc.Bacc("TRN2", target_bir_lowering=False)

    xt_d = nc.dram_tensor("xt", [B, C, N], fp16, kind="ExternalInput")
    xqt_d = nc.dram_tensor("xqt", [B, C, RQ], fp16, kind="ExternalInput")
    mt_d = nc.dram_tensor("maskst", [NCH, 128, 3, 128], fp16, kind="ExternalInput")
    wqt_d = nc.dram_tensor("wqt", [C, 64], fp16, kind="ExternalInput")
    wkt_d = nc.dram_tensor("wkt", [C, 64], fp16, kind="ExternalInput")
    wvt_d = nc.dram_tensor("wvt", [C, C], fp16, kind="ExternalInput")
    pcol_d = nc.dram_tensor("pcol", [64, 3, L], f32, kind="ExternalInput")
    out_d = nc.dram_tensor("out", [B, RQ, C], f32, kind="ExternalOutput")

    debug = bool(int(os.environ.get("KBENCH_DEBUG", "0")))
    if debug:
        dbg_w_d = nc.dram_tensor("dbg_w", [128, 3, L, RQ], f32,
                                 kind="ExternalOutput")
        dbg_scores_d = nc.dram_tensor("dbg_scores", [128, L, RQ], fp16,
                                      kind="ExternalOutput")
        dbg_probs_d = nc.dram_tensor("dbg_probs", [128, L, RQ], bf16,
                                     kind="ExternalOutput")

    with tile.TileContext(nc) as tc, tc.tile_pool(name="singles", bufs=1) as singles, \
            tc.tile_pool(name="xtb", bufs=2) as xtb_pool, \
            tc.tile_pool(name="small", bufs=3) as small, \
            tc.tile_pool(name="wsb", bufs=6) as wsb_pool, \
            tc.tile_pool(name="prod", bufs=5) as prod, \
            tc.tile_pool(name="probs", bufs=5) as probs_pool, \
            tc.tile_pool(name="epi", bufs=2) as epi, \
            tc.tile_pool(name="w_ps", bufs=3, space="PSUM") as w_ps_pool, \
            tc.tile_pool(name="pv_ps", bufs=1, space="PSUM") as pv_ps:

        # ---------------- resident tensors ----------------
        wqt = singles.tile([C, 64], fp16)
        wkt = singles.tile([C, 64], fp16)
        wvt = singles.tile([C, C], fp16)
        nc.sync.dma_start(out=wqt, in_=wqt_d[:, :])
        nc.sync.dma_start(out=wkt, in_=wkt_d[:, :])
        nc.sync.dma_start(out=wvt, in_=wvt_d[:, :])

        pcol = singles.tile([64, 3, L], f32)
        nc.sync.dma_start(out=pcol, in_=pcol_d[:, :, :])

        xqT = singles.tile([C, B, RQ], fp16)

        masksT = singles.tile([128, NCH, 3, 128], fp16)  # [j, ch, m, i]
        kT = singles.tile([64, B, N], fp16)
        qtb = singles.tile([64, B, 3, L, RQ], fp16)  # P-scaled q, 24 virtual heads
        v17 = singles.tile([128, B, NCH, L, 17], bf16)  # [j, ..., l, d|ones]

        # ones column of v17 (copies below fill [..,0:16])
        nc.gpsimd.memset(v17[:, :, :, :, 16:17], 1.0)

        # ---------------- per-batch projections ----------------
        def proj(b):
            xT = xtb_pool.tile([C, N], fp16, tag="xT", name="xT")  # x[b].T
            # halves: the kT matmul consumes xT per-half, so it can start
            # while the second half is still in flight
            for h in range(2):
                nc.sync.dma_start(out=xT[:, h * 512:(h + 1) * 512],
                                  in_=xt_d[b, :, h * 512:(h + 1) * 512])
            nc.sync.dma_start(out=xqT[:, b], in_=xqt_d[b])

            # kT[b] = wkt.T @ xT   [64, N]
            for h in range(2):
                ps = w_ps_pool.tile([64, 512], f32, tag="w", name="kps")
                nc.tensor.matmul(ps, wkt, xT[:, h * 512:(h + 1) * 512],
                                 start=True, stop=True)
                nc.any.tensor_copy(out=kT[:, b, h * 512:(h + 1) * 512], in_=ps)

            # qT[b] = wqt.T @ xqT[b] [64, RQ] -> P-scaled copies into qtb
            ps = w_ps_pool.tile([64, 512], f32, tag="w", name="qps")
            nc.tensor.matmul(ps[:, 0:RQ], wqt, xqT[:, b, :], start=True, stop=True)
            qt_sb = small.tile([64, RQ], fp16, tag="qt", name="qt_sb")
            nc.any.tensor_copy(out=qt_sb, in_=ps[:, 0:RQ])
            for m in range(3):
                eng = nc.gpsimd if m >= 1 else nc.vector
                for l in range(L):
                    eng.tensor_scalar_mul(
                        qtb[:, b, m, l, :], qt_sb, pcol[:, m, l, None])

            # v[b] chunk-by-chunk: v = x @ Wv.T  -> v17 (bf16, strided dst)
            for ch in range(NCH):
                ps = w_ps_pool.tile([128, 128], f32, tag="w", name="vps")
                nc.tensor.matmul(ps, xT[:, ch * 128:(ch + 1) * 128], wvt,
                                 start=True, stop=True)
                nc.scalar.copy(
                    out=v17[:, b, ch, :, 0:16],
                    in_=ps.rearrange("p (l d) -> p l d", l=L),
                )

        for b in range(B):
            proj(b)

        # masksT loads are only needed by the coupling stage; issuing them
        # after the projection prologue keeps the startup DMAs on xt/weights
        for ch in range(NCH):
            nc.sync.dma_start(out=masksT[:, ch], in_=mt_d[ch])

        # ---------------- main loop ----------------
        for b in range(B):
            pv = pv_ps.tile([17, L, RQ], f32)  # accumulated over ch
            for ch in range(NCH):
                # w[m,l][j,i] = sum_gd kT[gd,j] qtb[gd,(m,l,i)]  on PE.
                # w_sb is m-major so each psum->sbuf copy has a CONTIGUOUS
                # destination (keeps the DVE 2x packing mode); the coupling
                # mult reads the permuted [l,m,i] view (unit innermost, 2x ok)
                w_sb = wsb_pool.tile([128, 3, L, RQ], fp16, tag="wsb")
                for m in range(3):
                    wp = w_ps_pool.tile([128, L, RQ], f32, tag="w")
                    wpf = wp.rearrange("p l i -> p (l i)")
                    qf = qtb[:, b, m].rearrange("p l i -> p (l i)")
                    for h in range(2):  # psum bank limit: <=512 f32 per matmul
                        nc.tensor.matmul(
                            wpf[:, h * 512:(h + 1) * 512],
                            kT[:, b, ch * 128:(ch + 1) * 128],
                            qf[:, h * 512:(h + 1) * 512],
                            start=True, stop=True,
                        )
                    # copy w psum -> sbuf fp16 (2 on ACT, 1 on DVE)
                    if m < 2:
                        nc.scalar.copy(out=w_sb[:, m], in_=wp)
                    else:
                        nc.vector.tensor_copy(out=w_sb[:, m], in_=wp)
                    if debug and b == 0 and ch == 0:
                        nc.sync.dma_start(out=dbg_w_d[:, m], in_=wp)

                # coupling: scores_l = sum_m masksT[ch,m] (bcast l) * w[l,m]
                prm = prod.tile([128, L, 3, RQ], fp16, tag="prm")
                nc.vector.tensor_tensor(
                    out=prm,
                    in0=masksT[:, ch, None, :, :].to_broadcast((128, L, 3, RQ)),
                    in1=w_sb.rearrange("p m l i -> p l m i"),
                    op=OP.mult,
                )
                s12 = prod.tile([128, L, RQ], fp16, tag="s12")
                sc = prod.tile([128, L, RQ], fp16, tag="sc")
                # both adds on one engine per iteration: avoids a cross-engine
                # hop in the mult->add->add->exp chain
                eng = nc.vector if (b * NCH + ch) % 3 == 0 else nc.gpsimd
                eng.tensor_tensor(
                    out=s12, in0=prm[:, :, 1, :], in1=prm[:, :, 2, :], op=OP.add)
                eng.tensor_tensor(
                    out=sc, in0=prm[:, :, 0, :], in1=s12, op=OP.add)

                # probs = exp(scores - 10): global shift cancels in the
                # normalization exactly, keeps exp within fp16 range
                pb = probs_pool.tile([128, L, RQ], bf16, tag="probs")
                nc.scalar.activation(out=pb, in_=sc, func=AF.Exp)

                if debug and b == 0 and ch == 0:
                    nc.sync.dma_start(out=dbg_scores_d[:, :, :], in_=sc)
                    nc.sync.dma_start(out=dbg_probs_d[:, :, :], in_=pb)

                for l in range(L):
                    # start=True clears has_written for the WHOLE psum bank:
                    # only the first matmul touching each bank may set it
                    # (pv spans 2 banks: l 0-3 and l 4-7).
                    nc.tensor.matmul(
                        pv[:, l, :],
                        v17[:, b, ch, l, :],
                        pb[:, l, :],
                        start=(ch == 0 and l % 4 == 0), stop=(ch == NCH - 1),
                        skip_group_check=True,
                    )

            # epilogue: 32x32 block-transpose of pv, normalize, store.
            # tr[i%32, l, i//32, c] = pv[c, l, i]; row c=16 is the denom.
            pv_sb = epi.tile([32, L, RQ], bf16, tag="pvsb")
            nc.gpsimd.memset(pv_sb, 0.0)
            nc.scalar.copy(out=pv_sb[0:17], in_=pv)
            tr = epi.tile([32, L, 4, 32], bf16, tag="pvtr")
            nc.vector.transpose(
                out=tr.rearrange("p l k r -> p (l k r)"),
                in_=pv_sb.rearrange("p l i -> p (l i)"),
            )
            denr = epi.tile([32, L, 4], f32, tag="denr")
            nc.vector.reciprocal(out=denr, in_=tr[:, :, :, 16])
            ob = epi.tile([32, L, 4, 16], f32, tag="ob")
            nc.vector.tensor_tensor(
                out=ob,
                in0=tr[:, :, :, 0:16],
                in1=denr[:, :, :, None].to_broadcast((32, L, 4, 16)),
                op=OP.mult,
            )
            # out[b, kb*32+r, l*16+d] <- ob[r, l, kb, d]
            ob_dst = bass.AP(
                tensor=out_d, offset=b * RQ * C,
                ap=[[C, 32], [16, L], [32 * C, 4], [1, 16]],
            )
            nc.sync.dma_start(out=ob_dst, in_=ob)

    nc.compile()
    return nc


def _get_graph():
    if "nc" not in _cache:
        _cache["nc"] = _build()
    return _cache["nc"]


def kernel(x, masks, Wq, Wk, Wv, mask_proj):
    from concourse import bass_utils

    x = np.asarray(x, dtype=np.float32)
    masks = np.asarray(masks, dtype=np.float32)
    Wq = np.asarray(Wq, dtype=np.float32)
    Wk = np.asarray(Wk, dtype=np.float32)
    Wv = np.asarray(Wv, dtype=np.float32)
    mask_proj = np.asarray(mask_proj, dtype=np.float32)

    f16 = np.float16
    xt = np.ascontiguousarray(x.transpose(0, 2, 1)).astype(f16)  # [B, C, N]
    wqt = np.ascontiguousarray(Wq.T).astype(f16)
    wkt = np.ascontiguousarray(Wk.T).astype(f16)
    wvt = np.ascontiguousarray(Wv.T).astype(f16)
    # pcol[gd, m, l] = mask_proj[m, g(gd)*L + l]
    g_of = (np.arange(64) // HD)
    pcol = np.ascontiguousarray(
        mask_proj[None, :, :].repeat(64, 0)[
            np.arange(64)[:, None, None],
            np.arange(3)[None, :, None],
            (g_of[:, None, None] * L + np.arange(L)[None, None, :])]
    ).astype(np.float32)

    in_maps = []
    for r in range(NCORES):
        sl = slice(r * RQ, (r + 1) * RQ)
        # maskst[ch, j, m, i] = masks[r*128+i, ch*128+j, m]
        msl = masks[sl]  # [i=128, N, 3]
        mt = np.ascontiguousarray(
            msl.reshape(RQ, NCH, 128, 3).transpose(1, 2, 3, 0)).astype(f16)
        in_maps.append({
            "xt": xt,
            "xqt": np.ascontiguousarray(xt[:, :, sl]),
            "maskst": mt,
            "wqt": wqt, "wkt": wkt, "wvt": wvt, "pcol": pcol,
        })

    nc = _get_graph()
    trace = bool(int(os.environ.get("KBENCH_TRACE", "0")))
    try:
        res = bass_utils.run_bass_kernel_spmd(
            nc, in_maps, core_ids=list(range(NCORES)), trace=trace,
        )
    except (ImportError, ModuleNotFoundError):
        # NTFF profile hook unavailable in this environment; run untraced
        res = bass_utils.run_bass_kernel_spmd(
            nc, in_maps, core_ids=list(range(NCORES)), trace=False,
        )
    _cache["last_exec_time_ns"] = getattr(res, "exec_time_ns", None)

    out = np.empty((B, N, C), dtype=np.float32)
    for r in range(NCORES):
        out[:, r * RQ:(r + 1) * RQ, :] = res.results[r]["out"]
    return out

